# revision 58
# baseline (speedup 1.0000x reference)
"""MultiHeadAttention (tanh-capped logits, key-padding mask) on 8 Trainium2 cores.

Problem: B=4, S=2048, E=1024, H=16, DH=64.
  u = (Q K^T) * scale / sqrt(DH); logits = tanh(u) * exp(log_C)
  logits[masked] = -inf; attn = softmax(logits); out = (attn V) @ W_out.T

Sharding: core c handles batch b=c//2 and heads [8*(c%2), 8*(c%2)+8).
Each core computes a partial y^T = sum over its 8 heads of W_out-slice @ o_h^T;
the host sums the 2 cores of each batch and transposes.

Key optimizations over the naive mapping (672us -> ~270us on HW):
  * Masked keys are compacted away on the host (mask is per-(batch,key)):
    only ~half the keys survive, halving QK matmul, activation and PV work.
    Padding keys have zeroed V rows and zeroed ones-column, so softmax stays
    exact.
  * exp(C*tanh(u)) is computed as e^alpha*(s*(1+c1*s+c2*s^2))^4 with
    s = sigmoid(a*u-b): ONE ACT pass (sigmoid) plus one 8-stage custom DVE
    instruction, instead of two full ACT passes (tanh then exp). The
    constant e^alpha cancels in the softmax; the fit minimizes worst-case
    softmax-weight-ratio error (<1e-4 where weight is carried). This halves
    the former ACT bottleneck and splits it across two engines.
  * The W_O projection contracts head PAIRS (K=128 instead of 64): the odd
    head's normalized output is DMA-moved into partitions 64..127.
  * Heads are software-pipelined (QK/sigmoid/sigpow of head h+1 interleaved
    with PV groups of head h) and each q-tile's projection is spread two
    output-blocks at a time across the next q-tile's heads 1-4, so the PE
    gets steady work and mostly stays out of the HAM half-clock throttle.
  * PSUM->SBUF result copies run on ACT (which has headroom after losing
    the exp pass) so the DVE stays dedicated to the sigmoid-power op.
"""
import math
import os

os.environ.setdefault("JAX_COMPILATION_CACHE_DIR", "/tmp/jax_comp_cache")

import numpy as np

import concourse.bass as bass
import concourse.tile as tile
from concourse import bacc, mybir
from concourse import bass_utils
from concourse.bass_interp import get_hw_module

F32 = mybir.dt.float32
F32R = mybir.dt.float32r

# ---------------------------------------------------------------------------
# Custom DVE op: out = (s * (1 + c1*s + c2*s^2))^4  (8 ALU stages, 2 scalars)
#
# With s = sigmoid(a*u - b), suitable (a, b, c1, c2) make
# e^alpha * (s*(1+c1*s+c2*s^2))^4 match exp(C*tanh(u)) to ~1e-4 relative on
# the region that carries softmax weight (the constant e^alpha cancels in the
# softmax, and the low-u region only underestimates, which vanishes there).
# This replaces the Exp ACT pass entirely: one Sigmoid ACT pass + this DVE op.
# ---------------------------------------------------------------------------
from concourse import dve_ops as _dve_ops
from concourse.dve_spec import Spec as _Spec, Src0 as _Src0, C0 as _C0, C1 as _C1, One as _One, lower as _dve_lower, _has_src1
from concourse.dve_uop import DveOpSpec as _DveOpSpec


def _sigpow_reference(in0, in1, s0, s1, imm2):
    s = in0.astype(np.float32)
    base = (s * (1.0 + s0 * s + s1 * s * s)).astype(np.float32)
    b2 = (base * base).astype(np.float32)
    return (b2 * b2).astype(np.float32)


def _register_sigpow():
    name = "SIGPOW4_ANT"
    for o in _dve_ops.OPS:
        if o.name == name:
            return o
    s2 = _Src0 * _Src0
    inner = (_Src0 * _C0) + (s2 * _C1) + _One
    base = _Src0 * inner
    b2 = base * base
    spec = _Spec(body=b2 * b2, reference=_sigpow_reference)
    row = _dve_ops._CUSTOM_DVE_ROW_BASE + len(_dve_ops.OPS)
    shas = {}
    for ver in ("v3", "v4"):
        try:
            tmp = _DveOpSpec(
                name=name,
                opcode=row,
                uops=_dve_lower(spec, ver=ver),
                rd1_en=_has_src1(spec),
            )
            shas[ver] = tmp.sha(ver)
        except Exception:
            pass
    op = _dve_ops.DveOp(name, spec, subdim=False, uops_sha=shas)
    _dve_ops.OPS.append(op)
    _dve_ops.CUSTOM_DVE_SPECS[name] = spec
    _dve_ops._SUB_OPCODE_FOR_NAME[name] = row
    return op


_SIGPOW = _register_sigpow()

# fit of alpha + 4*(ln s + ln(1+c1 s+c2 s^2)), s=sigmoid(a*u-b) against
# 10*tanh(u), minimizing worst softmax-weight-ratio error (see module doc)
_SP_A = 2.0010366627048146
_SP_B = 0.572676623723485
_SP_C1 = -0.8183786895547329
_SP_C2 = 6.656049619653574

B, S, E, H, DH = 4, 2048, 1024, 16, 64
N_CORES = 8
HPC = 8  # heads per core
QT = 512  # q tile
NQT = S // QT  # 4

_CACHE = {}
LAST_RESULTS = None


def _round_f32r(x: np.ndarray) -> np.ndarray:
    """Round-to-nearest-even keeping 11 mantissa bits (hardware f32r format)."""
    u = x.astype(np.float32).view(np.uint32).astype(np.uint64)
    shift = 12  # 23 - 11
    bias = (1 << (shift - 1)) - 1
    u = (u + bias + ((u >> shift) & 1)) >> shift << shift
    return (u & 0xFFFFFFFF).astype(np.uint32).view(np.float32)


def _build(scale_eff: float, gain: float, nkt: int, sigpow: bool):
    """nkt: number of 128-key blocks after mask compaction (SK = nkt*128)."""
    SK = nkt * 128
    # kt groups of <=3 blocks: one 3-bank psum tile + one tanh instr per group
    groups = []
    k0 = 0
    while k0 < nkt:
        groups.append(list(range(k0, min(k0 + 3, nkt))))
        k0 += 3

    nc = bacc.Bacc(
        "TRN2",
        target_bir_lowering=False,
        debug=False,
        enable_asserts=True,
        num_devices=N_CORES,
    )
    kT_d = nc.dram_tensor("kT", [128, 4, SK], F32R, kind="ExternalInput").ap()
    qT_d = nc.dram_tensor("qT", [128, 4, S], F32R, kind="ExternalInput").ap()
    v_d = nc.dram_tensor("vA", [128, HPC, nkt, DH + 1], F32R, kind="ExternalInput").ap()
    # head-pair layout: partitions 0:64 = even head dh, 64:128 = odd head dh
    woT_d = nc.dram_tensor("woT", [128, HPC // 2, E], F32R, kind="ExternalInput").ap()
    yT_d = nc.dram_tensor("yT", [E, S], F32, kind="ExternalOutput").ap()

    with tile.TileContext(nc) as tc:
        with (
            tc.tile_pool(name="resident", bufs=1) as res_pool,
            tc.tile_pool(name="qstream", bufs=2) as q_pool,
            tc.tile_pool(name="pt", bufs=3) as pt_pool,
            tc.tile_pool(name="onorm", bufs=3) as onorm_pool,
            tc.tile_pool(name="oddtmp", bufs=3) as odd_pool,
            tc.tile_pool(name="rspool", bufs=2) as rs_pool,
            tc.tile_pool(name="yout", bufs=2) as y_pool,
            tc.tile_pool(name="qk_ps", bufs=2, space="PSUM") as qk_ps,
            tc.tile_pool(name="pv_ps", bufs=2, space="PSUM") as pv_ps,
        ):
            # Resident loads. First QK needs kT pair j=0 and the first q-tile:
            # those go first on the sync queue; the rest spread over queues.
            kT_sb = res_pool.tile([128, 4, SK], F32R, tag="kT")
            kmid = min(768, SK)
            nc.scalar.dma_start(out=kT_sb[:, 0, 0:384], in_=kT_d[:, 0, 0:384])
            nc.sync.dma_start(out=kT_sb[:, 0, 384:kmid], in_=kT_d[:, 0, 384:kmid])
            v_sb = res_pool.tile([128, HPC, nkt, DH + 1], F32R, tag="v")
            woT_sb = res_pool.tile([128, HPC // 2, E], F32R, tag="woT")
            # order matters: kT pair 1 is needed by head 2's QK a few us in,
            # so it must not queue behind the full V load
            nc.scalar.dma_start(out=kT_sb[:, 1], in_=kT_d[:, 1])
            nc.gpsimd.dma_start(out=v_sb[:, 0], in_=v_d[:, 0])
            nc.gpsimd.dma_start(out=v_sb[:, 1], in_=v_d[:, 1])
            for j in range(2, 4):
                nc.gpsimd.dma_start(out=kT_sb[:, j], in_=kT_d[:, j])
            for h in range(2, HPC):
                nc.gpsimd.dma_start(out=v_sb[:, h], in_=v_d[:, h])
            nc.gpsimd.dma_start(out=woT_sb, in_=woT_d)

            sig_bias = None
            if sigpow:
                sig_bias = res_pool.tile([128, 1], F32, tag="sigb")
                nc.vector.memset(sig_bias, -_SP_B)

            # preload the ACT table during the initial DMA wait instead of
            # paying the ~2.7us load on the first real activation
            warm = res_pool.tile([1, 8], F32, tag="warm")
            nc.vector.memset(warm, 0.0)
            nc.scalar.activation(
                out=warm,
                in_=warm,
                func=(
                    mybir.ActivationFunctionType.Sigmoid
                    if sigpow
                    else mybir.ActivationFunctionType.Tanh
                ),
            )

            def emit_proj_eo(onorm_pairs, qt_idx, eo, final=False):
                # projection borrows the qk psum ring (its tiles are a
                # superset of the [128, QT] needed here)
                py_full = qk_ps.tile([128, 3 * QT], F32, tag="qk", name="pyf")
                py = py_full[:, 0:QT]
                for p in range(HPC // 2):
                    nc.tensor.matmul(
                        py,
                        lhsT=woT_sb[:, p, eo * 128 : (eo + 1) * 128],
                        rhs=onorm_pairs[p],
                        start=(p == 0),
                        stop=(p == HPC // 2 - 1),
                    )
                y_t = y_pool.tile([128, QT], F32, tag="y")
                # ACT has headroom in the sigpow pipeline; DVE does not
                nc.scalar.copy(out=y_t, in_=py)
                # in the tail ACT is idle: use its queue for half the drain
                q_eng = (nc.sync, nc.scalar)[eo % 2] if final else nc.sync
                q_eng.dma_start(
                    out=yT_d[
                        eo * 128 : (eo + 1) * 128,
                        qt_idx * QT : (qt_idx + 1) * QT,
                    ],
                    in_=y_t,
                )

            def emit_proj(onorm_pairs, qt_idx, final=False):
                for eo in range(8):
                    emit_proj_eo(onorm_pairs, qt_idx, eo, final=final)

            pending = None  # (onorm_pairs, qt) awaiting projection emission

            def load_qt(qt):
                qT_t = q_pool.tile([128, 4, QT], F32R, tag="q", name="qT_t")
                nc.sync.dma_start(
                    out=qT_t[:, 0], in_=qT_d[:, 0, qt * QT : (qt + 1) * QT]
                )
                if qt == 0 and kmid < SK:
                    nc.sync.dma_start(
                        out=kT_sb[:, 0, kmid:SK], in_=kT_d[:, 0, kmid:SK]
                    )
                nc.sync.dma_start(
                    out=qT_t[:, 1:4], in_=qT_d[:, 1:4, qt * QT : (qt + 1) * QT]
                )
                return qT_t

            qT_next = load_qt(0)
            for qt in range(NQT):
                qT_t = qT_next
                onorm_pairs = [
                    onorm_pool.tile([128, QT], F32R, tag=f"on{p}", name=f"onp{p}")
                    for p in range(HPC // 2)
                ]
                pts = [None] * (HPC + 1)

                def emit_qk_group(h, g):
                    """QK matmuls + tanh + exp for head h, kt group g."""
                    j, half = h // 2, h % 2
                    lo = 64 * half
                    if g is groups[0]:
                        pts[h] = pt_pool.tile(
                            [128, nkt * QT], F32R, tag="pt", name="pt_t"
                        )
                    pt_t = pts[h]
                    ng = len(g)
                    ps = qk_ps.tile([128, 3 * QT], F32, tag="qk", name="ps")
                    for w, kt in enumerate(g):
                        nc.tensor.matmul(
                            ps[:, w * QT : (w + 1) * QT],
                            lhsT=kT_sb[lo : lo + 64, j, kt * 128 : (kt + 1) * 128],
                            rhs=qT_t[lo : lo + 64, j, :],
                            start=True,
                            stop=True,
                        )
                    sl = slice(g[0] * QT, (g[0] + ng) * QT)
                    if sigpow:
                        # one ACT pass: s = sigmoid(a*scale_eff*u - b), then
                        # the DVE computes (s*(1+c1 s+c2 s^2))^4 in place
                        nc.scalar.activation(
                            out=pt_t[:, sl],
                            in_=ps[:, 0 : ng * QT],
                            func=mybir.ActivationFunctionType.Sigmoid,
                            scale=_SP_A * scale_eff,
                            bias=sig_bias,
                        )
                        nc.vector._custom_dve(
                            _SIGPOW,
                            out=pt_t[:, sl],
                            in0=pt_t.bitcast(F32)[:, sl],
                            s0=_SP_C1,
                            s1=_SP_C2,
                        )
                    else:
                        nc.scalar.activation(
                            out=pt_t[:, sl],
                            in_=ps[:, 0 : ng * QT],
                            func=mybir.ActivationFunctionType.Tanh,
                            scale=scale_eff,
                        )
                        nc.scalar.activation(
                            out=pt_t[:, sl],
                            in_=pt_t[:, sl],
                            func=mybir.ActivationFunctionType.Exp,
                            scale=gain,
                        )

                def emit_pv_group(h, g, po):
                    """PV accumulation for head h over kt group g."""
                    pt_t = pts[h]
                    for kt in g:
                        nc.tensor.matmul(
                            po,
                            lhsT=v_sb[:, h, kt, :],
                            rhs=pt_t[:, kt * QT : (kt + 1) * QT],
                            start=(kt == 0),
                            stop=(kt == nkt - 1),
                        )

                def emit_norm(h, po):
                    """Normalize head h -> onorm_pairs[h//2] halves."""
                    pts[h] = None
                    # r lives on psum partition 64; engines are partition-
                    # locked, so copy it out at partition 64, DMA-move to
                    # partition 0, recip there, gpsimd-broadcast to 0-63.
                    rs = rs_pool.tile([DH + 1, QT], F32, tag="rs")
                    nc.scalar.copy(out=rs[DH : DH + 1, :], in_=po[DH : DH + 1, :])
                    mv = rs_pool.tile([1, QT], F32, tag="mv")
                    nc.scalar.dma_start(out=mv, in_=rs[DH : DH + 1, :])
                    rec1 = rs_pool.tile([1, QT], F32, tag="rec1")
                    nc.vector.reciprocal_approx_fast(out=rec1, in_=mv)
                    rb = rs_pool.tile([64, QT], F32, tag="rb")
                    nc.gpsimd.partition_broadcast(rb, rec1)
                    pair = onorm_pairs[h // 2]
                    if h % 2 == 0:
                        nc.vector.tensor_mul(
                            out=pair[0:DH, :], in0=po[0:DH, :], in1=rb
                        )
                    else:
                        tmp = odd_pool.tile([DH, QT], F32R, tag="odd")
                        nc.vector.tensor_mul(out=tmp, in0=po[0:DH, :], in1=rb)
                        nc.gpsimd.dma_start(out=pair[DH : 2 * DH, :], in_=tmp)

                # software pipeline: QK groups of head h+1 interleave with PV
                # groups of head h so the PE gets work in small chunks all
                # through head h's ACT window (avoids HAM throttle). The
                # previous q-tile's projection is spread 2 output-blocks at a
                # time across heads 1-4 for the same reason.
                # last q-tile: process the odd head of the final pair before
                # the even one, so the final projection's last dependency is
                # a direct DVE multiply instead of an SBUF->SBUF DMA
                final_qt = qt == NQT - 1
                order = [0, 1, 2, 3, 4, 5, 7, 6] if final_qt else list(range(HPC))
                for g in groups:
                    emit_qk_group(order[0], g)
                for hi, h in enumerate(order):
                    if hi == 6 and qt + 1 < NQT:
                        qT_next = load_qt(qt + 1)
                    po = pv_ps.tile([DH + 1, QT], F32, tag="pv", name="po")
                    for gi, g in enumerate(groups):
                        if hi + 1 < HPC:
                            emit_qk_group(order[hi + 1], groups[gi])
                        emit_pv_group(h, g, po)
                    if 1 <= hi <= 4 and pending is not None:
                        for eo in range(2 * (hi - 1), 2 * hi):
                            emit_proj_eo(pending[0], pending[1], eo)
                        if hi == 4:
                            pending = None
                    emit_norm(h, po)
                if final_qt:
                    emit_proj(onorm_pairs, qt, final=True)
                else:
                    pending = (onorm_pairs, qt)

    nc.compile()
    return nc


def _get_nc(scale_eff: float, gain: float, nkt: int):
    # the sigmoid-power fast path is fitted for cap C = exp(log_C) = 10;
    # fall back to exact tanh+exp for any other cap
    sigpow = abs(gain - 10.0) < 1e-6
    key = (round(scale_eff, 12), round(gain, 12), nkt, sigpow)
    if key not in _CACHE:
        _CACHE[key] = _build(scale_eff, gain, nkt, sigpow)
    return _CACHE[key]


def _prep_core_inputs(query, key, value, mask, W_out, nkt):
    """Host-side sharding + mask compaction + layout. Returns 8 in_maps."""
    SK = nkt * 128
    keep = ~mask[:, 0, :]  # [B, S]; True in mask = drop
    in_maps = []
    for c in range(N_CORES):
        b, hh = c // 2, c % 2
        hsl = slice(8 * hh, 8 * hh + 8)
        idx = np.nonzero(keep[b])[0]
        nk = len(idx)

        # compacted K: [SK, 8, 64], zero-padded beyond nk
        k4 = np.zeros((SK, HPC, DH), dtype=np.float32)
        k4[:nk] = key[b][idx].reshape(nk, H, DH)[:, hsl, :]
        kT = np.ascontiguousarray(
            k4.transpose(1, 2, 0).reshape(4, 128, SK).transpose(1, 0, 2)
        )
        q4 = query[b].reshape(S, H, DH)[:, hsl, :]
        qT = np.ascontiguousarray(
            q4.transpose(1, 2, 0).reshape(4, 128, S).transpose(1, 0, 2)
        )

        # compacted V augmented with ones column; padding rows stay zero so
        # they contribute nothing to either numerator or denominator
        aug = np.zeros((SK, HPC, DH + 1), dtype=np.float32)
        aug[:nk, :, :DH] = value[b][idx].reshape(nk, H, DH)[:, hsl, :]
        aug[:nk, :, DH] = 1.0
        vA = np.ascontiguousarray(
            aug.reshape(nkt, 128, HPC, DH + 1).transpose(1, 2, 0, 3)
        )

        # head-pair W_O^T layout: [128, 4, E]
        w4 = W_out.reshape(E, H, DH)[:, hsl, :]  # [E, 8, 64]
        woT = np.ascontiguousarray(
            w4.transpose(1, 2, 0).reshape(4, 128, E).transpose(1, 0, 2)
        )

        in_maps.append(
            {
                "kT": _round_f32r(kT),
                "qT": _round_f32r(qT),
                "vA": _round_f32r(vA),
                "woT": _round_f32r(woT),
            }
        )
    return in_maps


def kernel(query, key, value, mask, W_out, scale, log_C) -> np.ndarray:
    query = np.asarray(query, dtype=np.float32)
    key = np.asarray(key, dtype=np.float32)
    value = np.asarray(value, dtype=np.float32)
    mask = np.asarray(mask)
    W_out = np.asarray(W_out, dtype=np.float32)
    scale_eff = float(np.asarray(scale)) / math.sqrt(DH)
    gain = float(np.exp(np.float64(np.asarray(log_C))))

    keep = ~mask[:, 0, :]
    nkt = max(1, int(math.ceil(keep.sum(axis=1).max() / 128.0)))

    nc = _get_nc(scale_eff, gain, nkt)
    in_maps = _prep_core_inputs(query, key, value, mask, W_out, nkt)

    trace = os.environ.get("BASS_KERNEL_TRACE") == "1"
    old = nc.m
    nc.m = get_hw_module(nc.m)
    try:
        res = bass_utils.run_bass_kernel_spmd(
            nc, in_maps, core_ids=list(range(N_CORES)), trace=trace
        )
    finally:
        nc.m = old
    global LAST_RESULTS
    LAST_RESULTS = res

    out = np.empty((B, S, E), dtype=np.float32)
    for b in range(B):
        yT = res.results[2 * b]["yT"] + res.results[2 * b + 1]["yT"]
        out[b] = yT.T
    return out
